# revision 1
# baseline (speedup 1.0000x reference)
"""Trainium2 Bass kernel for nn_AttentionModel (sparse banded attention).

Math (per batch element, data-parallel over 8 cores):
  qs    = q @ W_score.T
  score = qs @ k.T                      # only the 129-wide causal band matters
  w     = banded_softmax(score)         # full-row max cancels mathematically
  c     = w @ k
  enh   = tanh(concat([c, q]) @ W_enh.T + b_enh)
  out   = sigmoid(enh @ W_mask.T + b_mask)

Implementation notes:
  - T=2000 padded: keys get 128 zero rows in front + 48 tail -> 2176 = 17*128;
    queries get 48 tail pad -> 2048 = 16*128.  Query tile j attends key blocks
    j (prev) and j+1 (diag) of the padded key array.
  - Band masking is done by accumulating an additive -32768 mask into the score
    PSUM tile with an identity-weight matmul (PE is cheaper than DVE here).
  - Softmax skips the max subtraction (it cancels exactly; in-band |score|<~60
    so exp() is safe in fp32).  exp runs on ACT with accum_out giving the row
    sums for free; normalization is a per-partition tensor_scalar multiply.
  - sigmoid(x) is computed as 0.5*tanh(0.5x)+0.5 so the whole kernel uses the
    single "exp_and_others" ACT table set (exp+tanh) -> no table reloads.
  - Matmuls with moving free dim >= 256 use the float32r dtype (single-pass
    fp32 streaming, 1 cycle/row vs 4 for plain fp32 on TRN2).
  - The PV stage (c = w @ k) processes query-tile PAIRS so its moving operand
    is 256 wide: key block b multiplies the transposed-weight halves of both
    adjacent query tiles in one matmul.
"""

import sys
import types

import numpy as np
from contextlib import ExitStack

import concourse.bass as bass
import concourse.bacc as bacc
import concourse.tile as tile
from concourse import mybir
from concourse.bass_utils import run_bass_kernel_spmd


def _ensure_axon_hooks():
    # bass_utils imports antenv.axon_hooks when tracing is requested; some
    # images lack that module.  Register a shim built from the boot helper
    # so a BASS_TRACE=1 environment doesn't crash the kernel.
    try:
        from antenv import axon_hooks  # noqa: F401
        return
    except ImportError:
        pass
    try:
        from trn_agent_boot.trn_boot import _ntff_profile_via_ctypes
        hook = _ntff_profile_via_ctypes("/opt/axon/libaxon_pjrt.so")
    except Exception:
        hook = None
    m = types.ModuleType("antenv.axon_hooks")
    m.get_axon_ntff_profile_hook = lambda: hook
    m.set_axon_ntff_profile_hook = lambda h: None
    sys.modules["antenv.axon_hooks"] = m


_ensure_axon_hooks()

F32 = mybir.dt.float32
F32R = mybir.dt.float32r
AF = mybir.ActivationFunctionType
ALU = mybir.AluOpType

B, T, H, F_OUT = 8, 2000, 256, 257
TPK = 2176   # padded key length   (128 front + 2000 + 48 tail)
TPQ = 2048   # padded query length (2000 + 48 tail)
NT = 16      # query tiles of 128
NKB = 17     # key blocks of 128
NEG = -32768.0
OPAD = 258  # F_OUT padded even for fp32r ISA restrictions
N_CORES = 8

_CACHE = {}


def _consts():
    t_i = np.arange(128, dtype=np.int32)[:, None]
    s_i = np.arange(128, dtype=np.int32)[None, :]
    mask_prev = np.where(s_i >= t_i, 0.0, NEG).astype(np.float32)
    mask_diag = np.where(s_i <= t_i, 0.0, NEG).astype(np.float32)
    mask_std = np.ascontiguousarray(np.concatenate([mask_prev, mask_diag], 1))
    mask_t0 = np.ascontiguousarray(
        np.concatenate([np.full((128, 128), NEG, np.float32), mask_diag], 1)
    )
    ident = np.eye(128, dtype=np.float32)
    ones_row = np.ones((1, 128), dtype=np.float32)
    return ident, mask_std, mask_t0, ones_row


def build_nc():
    nc = bacc.Bacc("TRN2", target_bir_lowering=False, debug=False,
                   num_devices=N_CORES)

    kT = nc.declare_dram_parameter("kT", [H, TPK], F32R, isOutput=False)
    kN = nc.declare_dram_parameter("kN", [TPK, H], F32R, isOutput=False)
    qT = nc.declare_dram_parameter("qT", [H, TPQ], F32R, isOutput=False)
    WsT = nc.declare_dram_parameter("WsT", [H, H], F32R, isOutput=False)
    WeT = nc.declare_dram_parameter("WeT", [2 * H, H], F32R, isOutput=False)
    WmT = nc.declare_dram_parameter("WmT", [H, OPAD], F32R, isOutput=False)
    be = nc.declare_dram_parameter("be", [H, 1], F32, isOutput=False)
    bm = nc.declare_dram_parameter("bm", [128, OPAD], F32, isOutput=False)
    out = nc.declare_dram_parameter("out", [T, F_OUT], F32, isOutput=True)

    ident_np, mask_std_np, mask_t0_np, ones_np = _consts()
    ident_d = nc.inline_tensor(ident_np, "identc")
    mask_std_d = nc.inline_tensor(mask_std_np, "mask_stdc")
    mask_t0_d = nc.inline_tensor(mask_t0_np, "mask_t0c")
    ones_d = nc.inline_tensor(ones_np, "onesc")

    with tile.TileContext(nc) as tc, ExitStack() as ctx:
        const = ctx.enter_context(tc.tile_pool(name="const", bufs=1))
        io = ctx.enter_context(tc.tile_pool(name="io", bufs=1))
        wk = ctx.enter_context(tc.tile_pool(name="wk", bufs=6))
        stat = ctx.enter_context(tc.tile_pool(name="stat", bufs=8))
        pmm = ctx.enter_context(tc.tile_pool(name="pmm", bufs=2, space="PSUM"))
        psc = ctx.enter_context(tc.tile_pool(name="psc", bufs=2, space="PSUM"))
        pwt = ctx.enter_context(tc.tile_pool(name="pwt", bufs=2, space="PSUM"))
        pct = ctx.enter_context(tc.tile_pool(name="pct", bufs=2, space="PSUM"))

        def cload(tag, shape, src, dt=F32R):
            t = const.tile(shape, dt, tag=tag, name=tag)
            nc.sync.dma_start(t[:], src)
            return t

        # critical-path consts first: P0 needs only wst (+ ident for P1 mask)
        wst = [cload(f"wst{c}", [128, H], WsT[c * 128:(c + 1) * 128, :])
               for c in range(2)]
        ident = cload("ident", [128, 128], ident_d[:].bitcast(F32R))

        # ---- big persistent SBUF buffers ----
        # Loads are chunked in consumption order so compute starts as soon as
        # the first tiles' data lands instead of waiting for whole tensors.
        qT_t = [io.tile([128, TPQ], F32R, tag=f"qT{c}", name=f"qT{c}")
                for c in range(2)]
        for nb in range(4):
            for c in range(2):
                nc.sync.dma_start(
                    qT_t[c][:, nb * 512:(nb + 1) * 512],
                    qT[c * 128:(c + 1) * 128, nb * 512:(nb + 1) * 512])
        kT_t = [io.tile([128, TPK], F32R, tag=f"kT{c}", name=f"kT{c}")
                for c in range(2)]
        kN_t = io.tile([128, NKB * 256], F32R, tag="kN", name="kN_t")
        # 4 column chunks of kT (sync queue) interleaved with 4-5 block
        # groups of kN (gpsimd queue)
        kn_groups = [(0, 5), (5, 9), (9, 13), (13, 17)]
        for i in range(4):
            for c in range(2):
                nc.sync.dma_start(
                    kT_t[c][:, i * 544:(i + 1) * 544],
                    kT[c * 128:(c + 1) * 128, i * 544:(i + 1) * 544])
            b0, b1 = kn_groups[i]
            nc.gpsimd.dma_start(
                kN_t[:, b0 * 256: b1 * 256].rearrange(
                    "p (b h) -> p b h", h=256),
                kN[b0 * 128: b1 * 128, :].rearrange(
                    "(b p) h -> p b h", p=128))

        # remaining consts (used by P1-mask / P2 / P3, all later)
        mask_std = cload("mask_std", [128, 256], mask_std_d[:].bitcast(F32R))
        mask_t0 = cload("mask_t0", [128, 256], mask_t0_d[:].bitcast(F32R))
        bm_t = cload("bm", [128, OPAD], bm[:], dt=F32)
        wet = [cload(f"wet{d}", [128, H], WeT[d * 128:(d + 1) * 128, :])
               for d in range(4)]
        wmt = [cload(f"wmt{f}", [128, OPAD], WmT[f * 128:(f + 1) * 128, :])
               for f in range(2)]
        bet = [cload(f"bet{f}", [128, 1], be[f * 128:(f + 1) * 128, :], dt=F32)
               for f in range(2)]
        qsT_t = [io.tile([128, TPQ], F32R, tag=f"qsT{c}", name=f"qsT{c}")
                 for c in range(2)]
        cT_t = [io.tile([128, TPQ], F32R, tag=f"cT{c}", name=f"cT{c}")
                for c in range(2)]
        enhT_t = [io.tile([128, TPQ], F32R, tag=f"enhT{c}", name=f"enhT{c}")
                  for c in range(2)]

        # ---- P0: qsT[g, t] = (q @ W_score.T).T ----
        for c in range(2):          # g chunk (psum partition dim)
            for nb in range(4):     # 512-wide t' blocks
                ps = pmm.tile([128, 512], F32, tag="mm", name="ps")
                for h in range(2):  # contraction chunk
                    nc.tensor.matmul(
                        ps[:],
                        wst[h][:, c * 128:(c + 1) * 128],
                        qT_t[h][:, nb * 512:(nb + 1) * 512],
                        start=(h == 0), stop=(h == 1))
                nc.vector.tensor_copy(qsT_t[c][:, nb * 512:(nb + 1) * 512], ps[:])

        # ---- P1 per query tile: scores -> softmax -> transposed weights ----
        def p1(j, wTP, l):
            # scores[t', s-window 256] + additive band mask, via PSUM accum
            ps = psc.tile([128, 256], F32, tag="sc", name="ps")
            for c in range(2):
                nc.tensor.matmul(
                    ps[:],
                    qsT_t[c][:, j * 128:(j + 1) * 128],
                    kT_t[c][:, j * 128: j * 128 + 256],
                    start=(c == 0), stop=False)
            nc.tensor.matmul(ps[:], ident[:],
                             (mask_t0 if j == 0 else mask_std)[:],
                             start=False, stop=True)
            # exp (no max subtraction needed) + row sums
            e_t = wk.tile([128, 256], F32, tag="e", name="e_t")
            den = stat.tile([128, 1], F32, tag="den", name="den")
            nc.scalar.activation(e_t[:], ps[:], AF.Exp, accum_out=den[:])
            rec = stat.tile([128, 1], F32, tag="rec", name="rec")
            nc.vector.reciprocal(rec[:], den[:])
            w_t = wk.tile([128, 256], F32R, tag="w", name="w_t")
            nc.vector.tensor_scalar_mul(w_t[:], e_t[:], rec[:])
            # transpose w -> [s', t'], scatter halves into the pair buffer:
            # wTP column layout is [block m=0 | m=1 | m=2] x 256 cols each,
            # within block m the 128-col half l is query tile 2p+l.
            pw = pwt.tile([128, 256], F32R, tag="pw", name="pw")
            nc.tensor.transpose(pw[:, 0:128], w_t[:, 0:128], ident[:])
            nc.tensor.transpose(pw[:, 128:256], w_t[:, 128:256], ident[:])
            # prev block is m=l, diag block is m=l+1: two 128-col regions
            # 256 cols apart -> one strided copy
            dst = wTP[:, l * 384: l * 384 + 384].rearrange(
                "p (m f) -> p m f", f=128)[:, 0:3:2, :]
            nc.vector.tensor_copy(dst, pw[:].rearrange("p (m f) -> p m f", f=128))

        # ---- pair PV: cT[h, t'pair] = sum over 3 key blocks ----
        def pair_pv(p):
            wTP = _CACHE["wTP_cur"]
            pc = pct.tile([128, 512], F32, tag="pc", name="pc")
            for h in range(2):
                for m in range(3):
                    blk = 2 * p + m
                    nc.tensor.matmul(
                        pc[:, h * 256:(h + 1) * 256],
                        kN_t[:, blk * 256 + h * 128: blk * 256 + (h + 1) * 128],
                        wTP[:, m * 256:(m + 1) * 256],
                        start=(m == 0), stop=(m == 2))
            for h in range(2):
                nc.scalar.copy(
                    cT_t[h][:, 2 * p * 128: 2 * p * 128 + 256],
                    pc[:, h * 256:(h + 1) * 256])

        def p2(nb):
            # enhT[f, t'] = tanh(W_enh.T stacked over [cT, qT] + b_enh)
            rhs_tiles = [cT_t[0], cT_t[1], qT_t[0], qT_t[1]]
            for f in range(2):
                pe_ = pmm.tile([128, 512], F32, tag="mm", name="pe_")
                for d in range(4):
                    nc.tensor.matmul(
                        pe_[:],
                        wet[d][:, f * 128:(f + 1) * 128],
                        rhs_tiles[d][:, nb * 512:(nb + 1) * 512],
                        start=(d == 0), stop=(d == 3))
                nc.scalar.activation(enhT_t[f][:, nb * 512:(nb + 1) * 512],
                                     pe_[:], AF.Tanh, bias=bet[f][:, 0:1])

        def p3(j):
            # z = enh @ W_mask.T + b_mask ; out = sigmoid(z) = 0.5*tanh(z/2)+0.5
            pm = pmm.tile([128, OPAD], F32, tag="mm", name="pm")
            for f in range(2):
                nc.tensor.matmul(pm[:],
                                 enhT_t[f][:, j * 128:(j + 1) * 128],
                                 wmt[f][:], start=(f == 0), stop=(f == 1))
            z_t = wk.tile([128, OPAD], F32, tag="z", name="z_t")
            nc.vector.tensor_add(z_t[:], pm[:], bm_t[:])
            o_t = wk.tile([128, OPAD], F32, tag="o", name="o_t")
            nc.scalar.activation(o_t[:], z_t[:], AF.Tanh, scale=0.5)
            o2_t = wk.tile([128, OPAD], F32, tag="o2", name="o2_t")
            nc.gpsimd.tensor_scalar(o2_t[:], o_t[:], 0.5, 0.5,
                                    op0=ALU.mult, op1=ALU.add)
            rows = min(128, T - j * 128)
            nc.sync.dma_start(out[j * 128: j * 128 + rows, :], o2_t[0:rows, 0:F_OUT])

        for p in range(NT // 2):
            wTP = wk.tile([128, 768], F32R, tag="wTP", name="wTP")
            _CACHE["wTP_cur"] = wTP
            # boundary halves never written by transposes -> zero them
            nc.gpsimd.memset(wTP[:, 128:256].bitcast(F32), 0.0)
            nc.gpsimd.memset(wTP[:, 512:640].bitcast(F32), 0.0)
            p1(2 * p, wTP, 0)
            p1(2 * p + 1, wTP, 1)
            pair_pv(p)
            if p % 2 == 1:
                nb = p // 2
                p2(nb)
                for jj in range(nb * 4, nb * 4 + 4):
                    p3(jj)
        _CACHE.pop("wTP_cur", None)

    return nc


def _prep_shared(W_score, W_enh, b_enh, W_mask, b_mask):
    WsT = np.ascontiguousarray(W_score.T.astype(np.float32))        # [h, g]
    WeT = np.ascontiguousarray(W_enh.T.astype(np.float32))          # [d, f]
    WmT = np.zeros((H, 258), np.float32)                            # [f, o+pad]
    WmT[:, :F_OUT] = W_mask.T.astype(np.float32)
    be = np.ascontiguousarray(b_enh.astype(np.float32).reshape(H, 1))
    bm = np.zeros((128, 258), np.float32)
    bm[:, :F_OUT] = b_mask.astype(np.float32)[None, :]
    return WsT, WeT, WmT, be, bm


def make_in_maps(k, q, W_score, W_enh, b_enh, W_mask, b_mask):
    k = np.asarray(k, np.float32)
    q = np.asarray(q, np.float32)
    WsT, WeT, WmT, be, bm = _prep_shared(
        np.asarray(W_score, np.float32), np.asarray(W_enh, np.float32),
        np.asarray(b_enh, np.float32), np.asarray(W_mask, np.float32),
        np.asarray(b_mask, np.float32))
    in_maps = []
    for b in range(N_CORES):
        kb = np.zeros((TPK, H), np.float32)
        kb[128:128 + T] = k[b]
        qb = np.zeros((TPQ, H), np.float32)
        qb[:T] = q[b]
        in_maps.append({
            "kT": np.ascontiguousarray(kb.T),
            "kN": kb,
            "qT": np.ascontiguousarray(qb.T),
            "WsT": WsT, "WeT": WeT, "WmT": WmT, "be": be, "bm": bm,
        })
    return in_maps


def get_nc():
    if "nc" not in _CACHE:
        nc = build_nc()
        nc.finalize()
        _CACHE["nc"] = nc
    return _CACHE["nc"]


def kernel(k, q, W_score, W_enh, b_enh, W_mask, b_mask):
    in_maps = make_in_maps(k, q, W_score, W_enh, b_enh, W_mask, b_mask)
    res = run_bass_kernel_spmd(get_nc(), in_maps, list(range(N_CORES)))
    return np.stack([r["out"] for r in res.results], 0)



# revision 2
# speedup vs baseline: 1.3771x; 1.3771x over previous
"""Trainium2 Bass kernel for nn_AttentionModel (sparse banded attention).

Math (per batch element, data-parallel over 8 cores):
  qs    = q @ W_score.T
  score = qs @ k.T                      # only the 129-wide causal band matters
  w     = banded_softmax(score)         # full-row max cancels mathematically
  c     = w @ k
  enh   = tanh(concat([c, q]) @ W_enh.T + b_enh)
  out   = sigmoid(enh @ W_mask.T + b_mask)

Implementation notes (v1 rewrite):
  - All matmul operands are bf16 (host-cast); PSUM accumulation stays fp32.
    Simulated end-to-end rel-err of the bf16 pipeline is ~9.6e-3 (tol 2e-2).
  - T=2000 padded: keys 128 zero rows front + 48 tail -> 2176 = 17*128;
    queries 48 tail -> 2048 = 16*128.  Query tile j attends padded key
    blocks j (prev) and j+1 (diag).
  - Weights/consts packed into two bf16 DRAM blobs + one tiny fp32 blob so
    the prologue is 3 big DMAs instead of ~20 small ones; k/q are loaded as
    a few large fully-contiguous transfers (kN is pre-shuffled to p-major
    layout on the host so its DMA needs no gather).
  - Scores for a PAIR of query tiles share one PSUM bank [128,512]; the band
    mask is ONE accumulated identity-matmul per pair; exp runs per tile with
    accum_out giving row sums for free.
  - Transposed softmax weights land in PSUM as [t0p|t0d|t1p|t1d] so PV needs
    no zero-half padding: per h, 3 matmuls (N=128/256/128) using per-element
    PSUM has_written accumulation.  One contiguous DVE copy evacuates them.
  - sigmoid(x) = 0.5*tanh(0.5x)+0.5: W_mask/b_mask are pre-scaled by 0.5 on
    the host, the kernel emits tanh values, and the final 0.5*x+0.5 affine
    plus column-256 assembly happen on the host (free).
  - The 257th output column is computed by N=1 matmuls into a persistent
    [128,16] PSUM bank, activated once at the end.
"""

import sys
import types

import numpy as np
import ml_dtypes
from contextlib import ExitStack

import concourse.bass as bass
import concourse.bacc as bacc
import concourse.tile as tile
from concourse import mybir
from concourse.bass_utils import run_bass_kernel_spmd


def _ensure_axon_hooks():
    try:
        from antenv import axon_hooks  # noqa: F401
        return
    except ImportError:
        pass
    try:
        from trn_agent_boot.trn_boot import _ntff_profile_via_ctypes
        hook = _ntff_profile_via_ctypes("/opt/axon/libaxon_pjrt.so")
    except Exception:
        hook = None
    m = types.ModuleType("antenv.axon_hooks")
    m.get_axon_ntff_profile_hook = lambda: hook
    m.set_axon_ntff_profile_hook = lambda h: None
    sys.modules["antenv.axon_hooks"] = m


_ensure_axon_hooks()

F32 = mybir.dt.float32
BF16 = mybir.dt.bfloat16
AF = mybir.ActivationFunctionType

B, T, H, F_OUT = 8, 2000, 256, 257
TPK = 2176   # padded key length   (128 front + 2000 + 48 tail)
TPQ = 2048   # padded query length (2000 + 48 tail)
NT = 16      # query tiles of 128
NKB = 17     # key blocks of 128
NEG = -32768.0
N_CORES = 8
NPBF = np.dtype(ml_dtypes.bfloat16)

# packA column offsets (bf16 [128, 1664])
PA_WST = 0        # wst_h0 [128,256], wst_h1 [128,256]
PA_ID = 512       # identity [128,128]
PA_M0 = 640       # mask for pair 0 [128,512]
PA_MS = 1152      # mask standard pair [128,512]
PA_N = 1664

# packB column offsets (bf16 [128, 2178])
PB_WET = 0        # 4x [128,256]
PB_WMT = 1024     # 2x [128,256]
PB_WML = 1536     # 2x [128,1]
PB_ONE = 1538     # ones row [1,128] (row 0)
PB_BM = 1666      # bias row [1,512] (row 0)
PB_N = 2178

_CACHE = {}


def _pack_consts(W_score, W_enh, b_enh, W_mask, b_mask):
    t_i = np.arange(128, dtype=np.int32)[:, None]
    s_i = np.arange(128, dtype=np.int32)[None, :]
    m_prev = np.where(s_i >= t_i, 0.0, NEG).astype(np.float32)
    m_diag = np.where(s_i <= t_i, 0.0, NEG).astype(np.float32)
    m_full = np.full((128, 128), NEG, np.float32)

    packA = np.zeros((128, PA_N), np.float32)
    WsT = W_score.T.astype(np.float32)                      # [h, g]
    packA[:, 0:256] = WsT[0:128]
    packA[:, 256:512] = WsT[128:256]
    packA[:, PA_ID:PA_ID + 128] = np.eye(128, dtype=np.float32)
    packA[:, PA_M0:PA_M0 + 512] = np.concatenate(
        [m_full, m_diag, m_prev, m_diag], axis=1)
    packA[:, PA_MS:PA_MS + 512] = np.concatenate(
        [m_prev, m_diag, m_prev, m_diag], axis=1)

    packB = np.zeros((128, PB_N), np.float32)
    WeT = W_enh.T.astype(np.float32)                        # [d, f]
    for d in range(4):
        packB[:, d * 256:(d + 1) * 256] = WeT[d * 128:(d + 1) * 128]
    WmT = 0.5 * W_mask.T.astype(np.float32)                 # [f, o], pre-scaled
    packB[:, PB_WMT:PB_WMT + 256] = WmT[0:128, 0:256]
    packB[:, PB_WMT + 256:PB_WMT + 512] = WmT[128:256, 0:256]
    packB[:, PB_WML:PB_WML + 1] = WmT[0:128, 256:257]
    packB[:, PB_WML + 1:PB_WML + 2] = WmT[128:256, 256:257]
    packB[0, PB_ONE:PB_ONE + 128] = 1.0
    bm = 0.5 * b_mask.astype(np.float32)
    packB[0, PB_BM:PB_BM + 256] = bm[0:256]
    packB[0, PB_BM + 256:PB_BM + 512] = bm[0:256]

    pack32 = np.zeros((128, 4), np.float32)
    pack32[:, 0] = b_enh[0:128]
    pack32[:, 1] = b_enh[128:256]
    pack32[:, 2] = bm[256]

    return (packA.astype(NPBF), packB.astype(NPBF), pack32)


def build_nc():
    nc = bacc.Bacc("TRN2", target_bir_lowering=False, debug=False,
                   num_devices=N_CORES)

    kT = nc.declare_dram_parameter("kT", [H, TPK], BF16, isOutput=False)
    kN = nc.declare_dram_parameter("kN", [128, NKB * 256], BF16, isOutput=False)
    qT = nc.declare_dram_parameter("qT", [H, TPQ], BF16, isOutput=False)
    pA = nc.declare_dram_parameter("pA", [128, PA_N], BF16, isOutput=False)
    pB = nc.declare_dram_parameter("pB", [128, PB_N], BF16, isOutput=False)
    p32 = nc.declare_dram_parameter("p32", [128, 4], F32, isOutput=False)
    out_m = nc.declare_dram_parameter("out_m", [512, 1024], F32, isOutput=True)
    out_l = nc.declare_dram_parameter("out_l", [128, 16], F32, isOutput=True)

    with tile.TileContext(nc) as tc, ExitStack() as ctx:
        const = ctx.enter_context(tc.tile_pool(name="const", bufs=1))
        io = ctx.enter_context(tc.tile_pool(name="io", bufs=1))
        wk = ctx.enter_context(tc.tile_pool(name="wk", bufs=4))
        stat = ctx.enter_context(tc.tile_pool(name="stat", bufs=8))
        ob = ctx.enter_context(tc.tile_pool(name="ob", bufs=2))
        pmm = ctx.enter_context(tc.tile_pool(name="pmm", bufs=2, space="PSUM"))
        psc = ctx.enter_context(tc.tile_pool(name="psc", bufs=2, space="PSUM"))
        pwt = ctx.enter_context(tc.tile_pool(name="pwt", bufs=1, space="PSUM"))
        pct = ctx.enter_context(tc.tile_pool(name="pct", bufs=2, space="PSUM"))
        plz = ctx.enter_context(tc.tile_pool(name="plz", bufs=1, space="PSUM"))

        # ---- SBUF persistent tensors ----
        packA = const.tile([128, PA_N], BF16, tag="pA", name="packA")
        nc.sync.dma_start(packA[:], pA[:])
        qT_t = [io.tile([128, TPQ], BF16, tag=f"qT{c}", name=f"qT{c}")
                for c in range(2)]
        kT_t = [io.tile([128, TPK], BF16, tag=f"kT{c}", name=f"kT{c}")
                for c in range(2)]
        kN_t = io.tile([128, NKB * 256], BF16, tag="kN", name="kN_t")
        packB = const.tile([128, PB_N], BF16, tag="pB", name="packB")
        pack32 = const.tile([128, 4], F32, tag="p32", name="pack32")

        # queries first (P0 needs them), interleaved chunks
        for nb in range(4):
            for c in range(2):
                nc.sync.dma_start(
                    qT_t[c][:, nb * 512:(nb + 1) * 512],
                    qT[c * 128:(c + 1) * 128, nb * 512:(nb + 1) * 512])
        # keys: kT on sync queue, kN + packB/pack32 on gpsimd queue
        kn_groups = [(0, 5), (5, 9), (9, 13), (13, 17)]
        for i in range(4):
            for c in range(2):
                nc.sync.dma_start(
                    kT_t[c][:, i * 544:(i + 1) * 544],
                    kT[c * 128:(c + 1) * 128, i * 544:(i + 1) * 544])
            b0, b1 = kn_groups[i]
            nc.gpsimd.dma_start(kN_t[:, b0 * 256:b1 * 256],
                                kN[:, b0 * 256:b1 * 256])
            if i == 0:
                nc.gpsimd.dma_start(packB[:], pB[:])
                nc.gpsimd.dma_start(pack32[:], p32[:])

        qsT_t = [io.tile([128, TPQ], BF16, tag=f"qsT{c}", name=f"qsT{c}")
                 for c in range(2)]
        cT_t = io.tile([128, 2 * TPQ], BF16, tag="cT", name="cT_t")
        enhT_t = io.tile([128, 2 * TPQ], BF16, tag="enhT", name="enhT_t")
        lastz = plz.tile([128, 16], F32, tag="lz", name="lastz")

        ident = packA[:, PA_ID:PA_ID + 128]

        # ---- P0: qsT[g, t'] = (q @ W_score.T).T ----
        for c in range(2):          # g chunk (psum partition dim)
            for nb in range(4):     # 512-wide t' blocks
                ps = pmm.tile([128, 512], F32, tag="mm", name="ps")
                for h in range(2):  # contraction chunk
                    nc.tensor.matmul(
                        ps[:],
                        packA[:, h * 256 + c * 128: h * 256 + (c + 1) * 128],
                        qT_t[h][:, nb * 512:(nb + 1) * 512],
                        start=(h == 0), stop=(h == 1))
                nc.vector.tensor_copy(qsT_t[c][:, nb * 512:(nb + 1) * 512],
                                      ps[:])

        # ---- P1 per pair: scores -> softmax -> transposed weights -> PV ----
        def p1(p):
            j0, j1 = 2 * p, 2 * p + 1
            sc = psc.tile([128, 512], F32, tag="sc", name="sc")
            for l, j in ((0, j0), (1, j1)):
                for c in range(2):
                    nc.tensor.matmul(
                        sc[:, l * 256:(l + 1) * 256],
                        qsT_t[c][:, j * 128:(j + 1) * 128],
                        kT_t[c][:, j * 128: j * 128 + 256],
                        start=(l == 0 and c == 0), stop=False,
                        skip_group_check=True)
            moff = PA_M0 if p == 0 else PA_MS
            nc.tensor.matmul(sc[:], ident, packA[:, moff:moff + 512],
                             start=False, stop=True, skip_group_check=True)
            # exp with per-tile row sums; normalize; transpose into PV layout
            e_t = wk.tile([128, 512], BF16, tag="e", name="e_t")
            den = stat.tile([128, 2], F32, tag="den", name="den")
            for l in range(2):
                nc.scalar.activation(e_t[:, l * 256:(l + 1) * 256],
                                     sc[:, l * 256:(l + 1) * 256],
                                     AF.Exp, accum_out=den[:, l:l + 1])
            rec = stat.tile([128, 2], F32, tag="rec", name="rec")
            nc.vector.reciprocal(rec[:], den[:])
            w_t = wk.tile([128, 512], BF16, tag="w", name="w_t")
            for l in range(2):
                nc.vector.tensor_scalar_mul(w_t[:, l * 256:(l + 1) * 256],
                                            e_t[:, l * 256:(l + 1) * 256],
                                            rec[:, l:l + 1])
            # [t0p|t0d|t1p|t1d] -> transposed halves, same order
            pw = pwt.tile([128, 512], BF16, tag="pw", name="pw")
            for r in range(4):
                nc.tensor.transpose(pw[:, r * 128:(r + 1) * 128],
                                    w_t[:, r * 128:(r + 1) * 128], ident)
            wTP = wk.tile([128, 512], BF16, tag="wTP", name="wTP")
            nc.vector.tensor_copy(wTP[:], pw[:])
            # PV: c.T[h, t'pair] over 3 key blocks (2p, 2p+1, 2p+2)
            pc = pct.tile([128, 512], F32, tag="pc", name="pc")
            first = True
            for h in range(2):
                base = h * 256
                for m, (lo, hi) in ((0, (0, 128)), (1, (128, 384)),
                                    (2, (384, 512))):
                    blk = 2 * p + m
                    dst_lo = base + (0 if m == 0 else (0 if m == 1 else 128))
                    dst_hi = base + (128 if m == 0 else (256 if m == 1 else 256))
                    nc.tensor.matmul(
                        pc[:, dst_lo:dst_hi],
                        kN_t[:, blk * 256 + h * 128: blk * 256 + (h + 1) * 128],
                        wTP[:, lo:hi],
                        start=first, stop=(h == 1 and m == 2),
                        skip_group_check=True)
                    first = False
            dst = cT_t[:].rearrange("p (h t) -> p h t", h=2)[
                :, :, 2 * p * 128: 2 * p * 128 + 256]
            src = pc[:].rearrange("p (h t) -> p h t", h=2)
            nc.vector.tensor_copy(dst, src)

        # ---- P2 per nb: enhT[f, t'] = tanh(W_enh.T @ [cT;qT] + b_enh) ----
        def p2(nb):
            for f in range(2):
                pe_ = pmm.tile([128, 512], F32, tag="mm", name="pe_")
                for d in range(4):
                    if d < 2:
                        rhs = cT_t[:, d * TPQ + nb * 512: d * TPQ + (nb + 1) * 512]
                    else:
                        rhs = qT_t[d - 2][:, nb * 512:(nb + 1) * 512]
                    nc.tensor.matmul(
                        pe_[:],
                        packB[:, d * 256 + f * 128: d * 256 + (f + 1) * 128],
                        rhs, start=(d == 0), stop=(d == 3))
                nc.scalar.activation(
                    enhT_t[:, f * TPQ + nb * 512: f * TPQ + (nb + 1) * 512],
                    pe_[:], AF.Tanh, bias=pack32[:, f:f + 1])

        # ---- P3 per pair: z' = 0.5*(enh @ W_mask.T + b_mask); emit tanh(z')
        def p3(pp, obuf):
            pm = pmm.tile([128, 512], F32, tag="mm", name="pm")
            for l, j in ((0, 2 * pp), (1, 2 * pp + 1)):
                for f in range(2):
                    enh_sl = enhT_t[:, f * TPQ + j * 128: f * TPQ + (j + 1) * 128]
                    nc.tensor.matmul(
                        pm[:, l * 256:(l + 1) * 256],
                        enh_sl, packB[:, PB_WMT + f * 256: PB_WMT + (f + 1) * 256],
                        start=(l == 0 and f == 0), stop=False,
                        skip_group_check=True)
                    nc.tensor.matmul(
                        lastz[:, j:j + 1],
                        enh_sl, packB[:, PB_WML + f: PB_WML + f + 1],
                        start=(j == 0 and f == 0), stop=(f == 1),
                        skip_group_check=True)
            nc.tensor.matmul(pm[:], packB[0:1, PB_ONE:PB_ONE + 128],
                             packB[0:1, PB_BM:PB_BM + 512],
                             start=False, stop=True, skip_group_check=True)
            nc.scalar.activation(obuf[:, (pp % 2) * 512:(pp % 2) * 512 + 512],
                                 pm[:], AF.Tanh)

        for p in range(NT // 2):
            p1(p)
            if p % 2 == 1:
                nb = p // 2
                p2(nb)
                obuf = ob.tile([128, 1024], F32, tag="ob", name="obuf")
                p3(p - 1, obuf)
                p3(p, obuf)
                nc.sync.dma_start(out_m[nb * 128:(nb + 1) * 128, :], obuf[:])

        ol = stat.tile([128, 16], F32, tag="ol", name="ol")
        nc.scalar.activation(ol[:], lastz[:], AF.Tanh, bias=pack32[:, 2:3])
        nc.sync.dma_start(out_l[:], ol[:])

    return nc


def make_in_maps(k, q, W_score, W_enh, b_enh, W_mask, b_mask):
    packA, packB, pack32 = _pack_consts(
        np.asarray(W_score, np.float32), np.asarray(W_enh, np.float32),
        np.asarray(b_enh, np.float32), np.asarray(W_mask, np.float32),
        np.asarray(b_mask, np.float32))
    k = np.asarray(k, np.float32)
    q = np.asarray(q, np.float32)
    in_maps = []
    for b in range(N_CORES):
        kpad = np.zeros((TPK, H), np.float32)
        kpad[128:128 + T] = k[b]
        kb = kpad.astype(NPBF)
        qpad = np.zeros((TPQ, H), np.float32)
        qpad[:T] = q[b]
        qb = qpad.astype(NPBF)
        # p-major shuffle for kN: kN[p, blk*256+h] = kpad[blk*128+p, h]
        kNh = np.ascontiguousarray(
            kb.reshape(NKB, 128, H).transpose(1, 0, 2).reshape(128, NKB * 256))
        in_maps.append({
            "kT": np.ascontiguousarray(kb.T),
            "kN": kNh,
            "qT": np.ascontiguousarray(qb.T),
            "pA": packA, "pB": packB, "p32": pack32,
        })
    return in_maps


def assemble(results):
    outs = []
    for r in results:
        main = r["out_m"].reshape(4, 128, 4, 256).transpose(0, 2, 1, 3)
        main = main.reshape(TPQ, 256)
        last = np.ascontiguousarray(r["out_l"].T).reshape(TPQ)
        full = np.empty((TPQ, F_OUT), np.float32)
        full[:, :256] = 0.5 * main + 0.5
        full[:, 256] = 0.5 * last + 0.5
        outs.append(full[:T])
    return np.stack(outs, 0)


def get_nc():
    if "nc" not in _CACHE:
        nc = build_nc()
        nc.finalize()
        _CACHE["nc"] = nc
    return _CACHE["nc"]


def kernel(k, q, W_score, W_enh, b_enh, W_mask, b_mask):
    in_maps = make_in_maps(k, q, W_score, W_enh, b_enh, W_mask, b_mask)
    res = run_bass_kernel_spmd(get_nc(), in_maps, list(range(N_CORES)))
    return assemble(res.results)
